# revision 38
# baseline (speedup 1.0000x reference)
"""AuxiliaryLossFreeRouter (MoE top-8 routing) on 8 Trainium2 NeuronCores.

Strategy (data-parallel over tokens, per the sharding hint):
  - 16384 tokens sharded 2048/core; gate_w + expert_bias replicated.
  - Gate matmul in bf16 hi/lo 3-pass split (xh@gh + xl@gh + xh@gl) accumulated
    in fp32 PSUM -> ~np.float32-level logits (max err ~2e-5) at 1 cycle/row
    per pass instead of fp32's 4 cycles/row. Same HBM bytes as fp32 (2x bf16).
    The xh pass streams a 512-wide rhs [gh|gl] into one PSUM bank (two
    accumulators side by side), folded by DVE adds that also apply the bias.
  - Per 128-token tile: DVE max8/max_index8 give top-8 values+indices,
    softmax over the 8 on ACT (Exp with accumulated sum) + DVE reciprocal,
    expert histogram via ones-vector matmul over a (logits >= v8) mask,
    accumulated in a dedicated PSUM bank across all tiles.
  - Host: unshard outputs, sum per-core histograms, derive scalar load stats.
"""

import os
import sys

if "/opt/trn_rl_repo" not in sys.path:
    sys.path.insert(0, "/opt/trn_rl_repo")

import ml_dtypes
import numpy as np

BF16 = ml_dtypes.bfloat16

N_CORES = 8
B, S, DM = 4, 4096, 2048
E = 256
K = 8
TOK = (B * S) // N_CORES  # 2048 tokens per core
NT = TOK // 128  # 16 token tiles per core
NK = DM // 128  # 16 contraction chunks

# packed const layout (bf16, [128, PK_W]): col 640 = ones column, the
# stationary operand of the histogram matmul (other cols unused)
PK_W = 641

_CACHE = {}
LAST_PROFILE = {}


def _build_program():
    if "nc" in _CACHE:
        return _CACHE["nc"]

    import concourse.tile as tile
    from concourse import bacc, mybir

    nc = bacc.Bacc("TRN2", target_bir_lowering=False, debug=False)

    xhl_d = nc.dram_tensor(
        "xhl", [NT, 128, 2 * NK * 128], mybir.dt.bfloat16, kind="ExternalInput"
    )
    ghl_d = nc.dram_tensor(
        "ghl", [128, NK * 2 * E], mybir.dt.bfloat16, kind="ExternalInput"
    )
    pk_d = nc.dram_tensor("pk", [128, PK_W], mybir.dt.bfloat16, kind="ExternalInput")
    biasb_d = nc.dram_tensor(
        "biasb", [128, E], mybir.dt.float32, kind="ExternalInput"
    )

    w_o = nc.dram_tensor("w_o", [128, NT * K], mybir.dt.float32, kind="ExternalOutput")
    idx_o = nc.dram_tensor(
        "idx_o", [128, NT * K], mybir.dt.uint32, kind="ExternalOutput"
    )
    counts_o = nc.dram_tensor(
        "counts_o", [1, 2 * E], mybir.dt.float32, kind="ExternalOutput"
    )

    f32 = mybir.dt.float32
    bf = mybir.dt.bfloat16

    with tile.TileContext(nc) as tc:
        with (
            tc.tile_pool(name="const", bufs=1) as cpool,
            tc.tile_pool(name="xin", bufs=6) as xpool,
            tc.tile_pool(name="work", bufs=3) as wpool,
            tc.tile_pool(name="acc", bufs=1) as apool,
            tc.tile_pool(name="lps", bufs=6, space="PSUM") as lpool,
            tc.tile_pool(name="cps", bufs=1, space="PSUM") as cpps,
        ):
            # DMA issue order = completion order on the single HWDGE queue.
            # x tile 0 goes LAST in the preamble: the PE's first matmul gates
            # on it, and by then weights + x tile 1 are resident, so the PE
            # stream never starves afterwards (a mid-stream gap re-throttles
            # the PE clock to 1.2 GHz for ~2 windows, costing far more).
            HG = NK * E  # half of the ghl columns
            ghl_s = cpool.tile([128, NK * 2 * E], bf)
            nc.scalar.dma_start(ghl_s[:, 0:HG], ghl_d.ap()[:, 0:HG])
            g_s = [ghl_s[:, k * 2 * E : (k + 1) * 2 * E] for k in range(NK)]

            def load_x_tile(t):
                xt = xpool.tile([128, 2 * NK * 128], bf, tag="x")
                nc.sync.dma_start(xt[:], xhl_d.ap()[t])
                return xt

            nc.scalar.dma_start(ghl_s[:, HG : 2 * HG], ghl_d.ap()[:, HG : 2 * HG])
            xt0 = load_x_tile(0)
            xt1 = load_x_tile(1)
            xt_pre = [xt0, xt1]

            # consts are first needed ~1 tile in; keep them off the
            # critical x/weight path
            pk_s = cpool.tile([128, PK_W], bf)
            nc.sync.dma_start(pk_s[:], pk_d.ap())
            bias_s = cpool.tile([128, E], f32)
            nc.sync.dma_start(bias_s[:], biasb_d.ap())

            w_acc = apool.tile([128, NT * K], f32)
            idx_acc = apool.tile([128, NT * K], mybir.dt.uint32)
            counts_p = cpps.tile([1, 2 * E], f32)
            masks = []

            # HAM warm-up: the PE idles ~15us waiting for the first DMAs and
            # would run the first ~3.4us of real matmuls at 1.2 GHz. Spin
            # no-dep dummy matmuls on scratch SBUF to lift the clock gate to
            # 2.4 GHz and keep it there until the real stream starts.
            warm_in = cpool.tile([128, 512], bf)
            nc.vector.memset(warm_in[:], 0.0)
            warm_p = cpps.tile([128, 512], f32, tag="warm")
            for _ in range(68):
                nc.tensor.matmul(
                    warm_p[:, 0:256],
                    warm_in[:, 0:128],
                    warm_in[:, 0:256],
                    start=True,
                    stop=True,
                )

            for t in range(NT):
                if t < len(xt_pre):
                    xt = xt_pre[t]
                else:
                    xt = load_x_tile(t)

                # xh pass with wide rhs: lp[:, 0:256] accumulates xh@gh,
                # lp[:, 256:512] accumulates xh@gl; k=0 clears the bank
                lp = lpool.tile([128, 2 * E], f32, tag="lp")
                for k in range(NK):
                    nc.tensor.matmul(
                        lp[:],
                        xt[:, k * 128 : (k + 1) * 128],
                        g_s[k][:],
                        start=(k == 0),
                        stop=False,
                    )
                # xl @ gh pass into the gh accumulator
                for k in range(NK):
                    nc.tensor.matmul(
                        lp[:, 0:E],
                        xt[:, (NK + k) * 128 : (NK + k + 1) * 128],
                        g_s[k][:, 0:E],
                        start=False,
                        stop=(k == NK - 1),
                    )

                # logits = (xh@gh + xl@gh) + (xh@gl + bias); DVE reads at
                # most one PSUM operand per op, so sum in two steps
                glpart = wpool.tile([128, E], f32, tag="glpart")
                nc.vector.tensor_add(glpart[:], bias_s[:], lp[:, E : 2 * E])
                logits = wpool.tile([128, E], f32, tag="logits")
                nc.vector.tensor_add(logits[:], glpart[:], lp[:, 0:E])

                vals8 = wpool.tile([128, 8], f32, tag="vals8")
                nc.vector.max(out=vals8[:], in_=logits[:])
                nc.vector.max_index(
                    out=idx_acc[:, t * K : (t + 1) * K],
                    in_max=vals8[:],
                    in_values=logits[:],
                )

                # two tiles' masks sit side by side in one [128, 512] tile so
                # the histogram matmul runs once per pair at N=512
                if t % 2 == 0:
                    mask2 = wpool.tile([128, 2 * E], bf, tag="mask")
                    masks.append(mask2)
                nc.vector.tensor_scalar(
                    masks[t // 2][:, (t % 2) * E : (t % 2 + 1) * E],
                    logits[:],
                    vals8[:, 7:8],
                    None,
                    op0=mybir.AluOpType.is_ge,
                )
                # the counts matmul for the previous pair: both its masks are
                # long done, so the in-order PE queue doesn't stall on the DVE
                if t >= 2 and t % 2 == 0:
                    nc.tensor.matmul(
                        counts_p[:],
                        pk_s[:, 640:641],
                        masks[t // 2 - 1][:],
                        start=(t == 2),
                        stop=False,
                        skip_group_check=True,
                    )

                negmax = wpool.tile([128, 1], f32, tag="negmax")
                nc.vector.tensor_scalar_mul(negmax[:], vals8[:, 0:1], -1.0)
                exp8 = wpool.tile([128, 8], f32, tag="exp8")
                sumexp = wpool.tile([128, 1], f32, tag="sumexp")
                nc.scalar.activation(
                    exp8[:],
                    vals8[:],
                    mybir.ActivationFunctionType.Exp,
                    bias=negmax[:],
                    scale=1.0,
                    accum_out=sumexp[:],
                )
                rsum = wpool.tile([128, 1], f32, tag="rsum")
                nc.vector.reciprocal(rsum[:], sumexp[:])
                nc.vector.tensor_scalar_mul(
                    w_acc[:, t * K : (t + 1) * K], exp8[:], rsum[:]
                )

            nc.tensor.matmul(
                counts_p[:],
                pk_s[:, 640:641],
                masks[NT // 2 - 1][:],
                start=False,
                stop=True,
                skip_group_check=True,
            )
            counts_s = apool.tile([1, 2 * E], f32)
            nc.vector.tensor_copy(counts_s[:], counts_p[:])

            nc.sync.dma_start(w_o.ap(), w_acc[:])
            nc.sync.dma_start(idx_o.ap(), idx_acc[:])
            nc.sync.dma_start(counts_o.ap(), counts_s[:])

    nc.compile()
    _CACHE["nc"] = nc
    return nc


def _install_trace_shim():
    """Enable NTFF profiling under axon (only used when KERNEL_TRACE=1)."""
    try:
        import types

        if "antenv.axon_hooks" in sys.modules:
            return True
        import antenv

        mod = types.ModuleType("antenv.axon_hooks")
        mod._hook = None
        mod.set_axon_ntff_profile_hook = lambda h: setattr(mod, "_hook", h)
        mod.get_axon_ntff_profile_hook = lambda: mod._hook
        sys.modules["antenv.axon_hooks"] = mod
        antenv.axon_hooks = mod
        from trn_agent_boot.trn_boot import _ntff_profile_via_ctypes

        mod._hook = _ntff_profile_via_ctypes("/opt/axon/libaxon_pjrt.so")
        from concourse import bass_utils

        bass_utils.upload_artifacts = lambda tmpdir: tmpdir
        return True
    except Exception:
        return False


def _prep_core_inputs(x_shard_f32):
    """x_shard [2048, 2048] f32 -> xhl [NT, 128, 2*NK*128] bf16.

    xhl[t, p, half*2048 + k*128 + tt] = half(x_shard[128*t + tt, 128*k + p])
    so each SBUF x-tile is [d-row partition, (half, chunk, token)] and chunk k
    of half h is the ready-to-use matmul lhsT [128 d, 128 tokens].
    """
    xh = x_shard_f32.astype(BF16)
    xl = (x_shard_f32 - xh.astype(np.float32)).astype(BF16)
    out = np.empty((NT, 128, 2 * NK * 128), BF16)
    for half, arr in enumerate((xh, xl)):
        # [NT, 128 tok, NK, 128 p] -> [NT, p, k, tok]
        r = arr.reshape(NT, 128, NK, 128).transpose(0, 3, 2, 1)
        out[:, :, half * NK * 128 : (half + 1) * NK * 128] = r.reshape(
            NT, 128, NK * 128
        )
    return out


def _prep_shared_inputs(gate_w, expert_bias):
    gw = np.ascontiguousarray(gate_w.T)  # [DM, E] f32
    gh = gw.astype(BF16)
    gl = (gw - gh.astype(np.float32)).astype(BF16)
    # [NK, 128 p, half, E]: per-chunk tile is [128 d-rows, gh | gl]
    ghl = np.stack(
        [g.reshape(NK, 128, E) for g in (gh, gl)], axis=2
    )  # [NK, 128, 2, E]
    # partition-major for one full-bandwidth DMA: [128, (k, half, e)]
    ghl = np.ascontiguousarray(ghl.transpose(1, 0, 2, 3).reshape(128, NK * 2 * E))

    pk = np.zeros((128, PK_W), BF16)
    pk[:, 2 * E + 128] = BF16(1.0)
    biasb = np.ascontiguousarray(
        np.broadcast_to(expert_bias.astype(np.float32), (128, E))
    )
    return ghl, pk, biasb


def kernel(x, gate_w, expert_bias):
    from concourse.bass_utils import run_bass_kernel_spmd

    x = np.asarray(x, np.float32)
    gate_w = np.asarray(gate_w, np.float32)
    expert_bias = np.asarray(expert_bias, np.float32)

    xf = x.reshape(B * S, DM)
    ghl, pk, biasb = _prep_shared_inputs(gate_w, expert_bias)

    in_maps = []
    for c in range(N_CORES):
        xhl = _prep_core_inputs(xf[c * TOK : (c + 1) * TOK])
        in_maps.append({"xhl": xhl, "ghl": ghl, "pk": pk, "biasb": biasb})

    nc = _build_program()

    trace = os.environ.get("KERNEL_TRACE", "") == "1"
    if trace:
        trace = _install_trace_shim()

    res = run_bass_kernel_spmd(
        nc, in_maps, core_ids=list(range(N_CORES)), trace=trace
    )
    LAST_PROFILE["exec_time_ns"] = res.exec_time_ns
    LAST_PROFILE["mean_exec_time_ns"] = res.mean_exec_time_ns
    LAST_PROFILE["trace"] = res.instructions_and_trace

    weights = np.empty((B * S, K), np.float32)
    indices = np.empty((B * S, K), np.int32)
    counts = np.zeros(E, np.float64)
    for c, out in enumerate(res.results):
        # [128 tok-in-tile, NT, K] -> [NT, 128, K] -> [2048, K]
        w = out["w_o"].reshape(128, NT, K).transpose(1, 0, 2).reshape(TOK, K)
        ix = out["idx_o"].reshape(128, NT, K).transpose(1, 0, 2).reshape(TOK, K)
        weights[c * TOK : (c + 1) * TOK] = w
        indices[c * TOK : (c + 1) * TOK] = ix.astype(np.int32)
        cc = out["counts_o"].ravel().astype(np.float64)
        counts += cc[:E] + cc[E:]

    expert_counts = counts.astype(np.float32)
    n_tokens = B * S * K
    expected_load = np.float32(n_tokens / E)
    mean = expert_counts.mean(dtype=np.float64)
    std = np.std(expert_counts.astype(np.float64), ddof=1)
    load_balance = np.float32(std / (mean + 1e-6))

    return (
        weights.reshape(B, S, K),
        indices.reshape(B, S, K),
        expert_counts,
        load_balance,
        np.float32(expert_counts.max()),
        np.float32(expert_counts.min()),
        expected_load,
    )


# revision 39
# speedup vs baseline: 1.0244x; 1.0244x over previous
"""AuxiliaryLossFreeRouter (MoE top-8 routing) on 8 Trainium2 NeuronCores.

Strategy (data-parallel over tokens, per the sharding hint):
  - 16384 tokens sharded 2048/core; gate_w + expert_bias replicated.
  - Gate matmul in bf16 hi/lo 3-pass split (xh@gh + xl@gh + xh@gl) accumulated
    in fp32 PSUM -> ~np.float32-level logits (max err ~2e-5) at 1 cycle/row
    per pass instead of fp32's 4 cycles/row. Same HBM bytes as fp32 (2x bf16).
    The xh pass streams a 512-wide rhs [gh|gl] into one PSUM bank (two
    accumulators side by side), folded by DVE adds that also apply the bias.
  - Per 128-token tile: DVE max8/max_index8 give top-8 values+indices,
    softmax over the 8 on ACT (Exp with accumulated sum) + DVE reciprocal,
    expert histogram via ones-vector matmul over a (logits >= v8) mask,
    accumulated in a dedicated PSUM bank across all tiles.
  - Host: unshard outputs, sum per-core histograms, derive scalar load stats.
"""

import os
import sys

if "/opt/trn_rl_repo" not in sys.path:
    sys.path.insert(0, "/opt/trn_rl_repo")

import ml_dtypes
import numpy as np

BF16 = ml_dtypes.bfloat16

N_CORES = 8
B, S, DM = 4, 4096, 2048
E = 256
K = 8
TOK = (B * S) // N_CORES  # 2048 tokens per core
NT = TOK // 128  # 16 token tiles per core
NK = DM // 128  # 16 contraction chunks

# packed const layout (bf16, [128, PK_W]): col 640 = ones column, the
# stationary operand of the histogram matmul (other cols unused)
PK_W = 641

_CACHE = {}
LAST_PROFILE = {}


def _build_program():
    if "nc" in _CACHE:
        return _CACHE["nc"]

    import concourse.tile as tile
    from concourse import bacc, mybir

    nc = bacc.Bacc("TRN2", target_bir_lowering=False, debug=False)

    xhl_d = nc.dram_tensor(
        "xhl", [NT, 128, 2 * NK * 128], mybir.dt.bfloat16, kind="ExternalInput"
    )
    ghl_d = nc.dram_tensor(
        "ghl", [128, NK * 2 * E], mybir.dt.bfloat16, kind="ExternalInput"
    )
    pk_d = nc.dram_tensor("pk", [128, PK_W], mybir.dt.bfloat16, kind="ExternalInput")
    biasb_d = nc.dram_tensor(
        "biasb", [128, E], mybir.dt.float32, kind="ExternalInput"
    )

    w_o = nc.dram_tensor("w_o", [128, NT * K], mybir.dt.float32, kind="ExternalOutput")
    idx_o = nc.dram_tensor(
        "idx_o", [128, NT * K], mybir.dt.uint32, kind="ExternalOutput"
    )
    counts_o = nc.dram_tensor(
        "counts_o", [1, 2 * E], mybir.dt.float32, kind="ExternalOutput"
    )

    f32 = mybir.dt.float32
    bf = mybir.dt.bfloat16

    with tile.TileContext(nc) as tc:
        with (
            tc.tile_pool(name="const", bufs=1) as cpool,
            tc.tile_pool(name="xin", bufs=6) as xpool,
            tc.tile_pool(name="work", bufs=3) as wpool,
            tc.tile_pool(name="acc", bufs=1) as apool,
            tc.tile_pool(name="lps", bufs=6, space="PSUM") as lpool,
            tc.tile_pool(name="cps", bufs=1, space="PSUM") as cpps,
        ):
            # DMA issue order = completion order on the single HWDGE queue.
            # x tile 0 goes LAST in the preamble: the PE's first matmul gates
            # on it, and by then weights + x tile 1 are resident, so the PE
            # stream never starves afterwards (a mid-stream gap re-throttles
            # the PE clock to 1.2 GHz for ~2 windows, costing far more).
            HG = NK * E  # half of the ghl columns
            ghl_s = cpool.tile([128, NK * 2 * E], bf)
            nc.sync.dma_start(ghl_s[:, 0:HG], ghl_d.ap()[:, 0:HG])
            g_s = [ghl_s[:, k * 2 * E : (k + 1) * 2 * E] for k in range(NK)]

            def load_x_tile(t):
                xt = xpool.tile([128, 2 * NK * 128], bf, tag="x")
                nc.sync.dma_start(xt[:], xhl_d.ap()[t])
                return xt

            nc.sync.dma_start(ghl_s[:, HG : 2 * HG], ghl_d.ap()[:, HG : 2 * HG])
            xt0 = load_x_tile(0)
            xt1 = load_x_tile(1)
            xt_pre = [xt0, xt1]

            # consts are first needed ~1 tile in; keep them off the
            # critical x/weight path
            pk_s = cpool.tile([128, PK_W], bf)
            nc.sync.dma_start(pk_s[:], pk_d.ap())
            bias_s = cpool.tile([128, E], f32)
            nc.sync.dma_start(bias_s[:], biasb_d.ap())

            w_acc = apool.tile([128, NT * K], f32)
            idx_acc = apool.tile([128, NT * K], mybir.dt.uint32)
            counts_p = cpps.tile([1, 2 * E], f32)
            masks = []

            # HAM warm-up: the PE idles ~15us waiting for the first DMAs and
            # would run the first ~3.4us of real matmuls at 1.2 GHz. Spin
            # no-dep dummy matmuls on scratch SBUF to lift the clock gate to
            # 2.4 GHz and keep it there until the real stream starts.
            warm_in = cpool.tile([128, 512], bf)
            nc.vector.memset(warm_in[:], 0.0)
            warm_p = cpps.tile([128, 512], f32, tag="warm")
            for _ in range(68):
                nc.tensor.matmul(
                    warm_p[:, 0:256],
                    warm_in[:, 0:128],
                    warm_in[:, 0:256],
                    start=True,
                    stop=True,
                )

            for t in range(NT):
                if t < len(xt_pre):
                    xt = xt_pre[t]
                else:
                    xt = load_x_tile(t)

                # xh pass with wide rhs: lp[:, 0:256] accumulates xh@gh,
                # lp[:, 256:512] accumulates xh@gl; k=0 clears the bank
                lp = lpool.tile([128, 2 * E], f32, tag="lp")
                for k in range(NK):
                    nc.tensor.matmul(
                        lp[:],
                        xt[:, k * 128 : (k + 1) * 128],
                        g_s[k][:],
                        start=(k == 0),
                        stop=False,
                    )
                # xl @ gh pass into the gh accumulator
                for k in range(NK):
                    nc.tensor.matmul(
                        lp[:, 0:E],
                        xt[:, (NK + k) * 128 : (NK + k + 1) * 128],
                        g_s[k][:, 0:E],
                        start=False,
                        stop=(k == NK - 1),
                    )

                # logits = (xh@gh + xl@gh) + (xh@gl + bias); DVE reads at
                # most one PSUM operand per op, so sum in two steps
                glpart = wpool.tile([128, E], f32, tag="glpart")
                nc.vector.tensor_add(glpart[:], bias_s[:], lp[:, E : 2 * E])
                logits = wpool.tile([128, E], f32, tag="logits")
                nc.vector.tensor_add(logits[:], glpart[:], lp[:, 0:E])

                vals8 = wpool.tile([128, 8], f32, tag="vals8")
                nc.vector.max(out=vals8[:], in_=logits[:])
                nc.vector.max_index(
                    out=idx_acc[:, t * K : (t + 1) * K],
                    in_max=vals8[:],
                    in_values=logits[:],
                )

                # two tiles' masks sit side by side in one [128, 512] tile so
                # the histogram matmul runs once per pair at N=512
                if t % 2 == 0:
                    mask2 = wpool.tile([128, 2 * E], bf, tag="mask")
                    masks.append(mask2)
                nc.vector.tensor_scalar(
                    masks[t // 2][:, (t % 2) * E : (t % 2 + 1) * E],
                    logits[:],
                    vals8[:, 7:8],
                    None,
                    op0=mybir.AluOpType.is_ge,
                )
                # the counts matmul for the previous pair: both its masks are
                # long done, so the in-order PE queue doesn't stall on the DVE
                if t >= 2 and t % 2 == 0:
                    nc.tensor.matmul(
                        counts_p[:],
                        pk_s[:, 640:641],
                        masks[t // 2 - 1][:],
                        start=(t == 2),
                        stop=False,
                        skip_group_check=True,
                    )

                negmax = wpool.tile([128, 1], f32, tag="negmax")
                nc.vector.tensor_scalar_mul(negmax[:], vals8[:, 0:1], -1.0)
                exp8 = wpool.tile([128, 8], f32, tag="exp8")
                sumexp = wpool.tile([128, 1], f32, tag="sumexp")
                nc.scalar.activation(
                    exp8[:],
                    vals8[:],
                    mybir.ActivationFunctionType.Exp,
                    bias=negmax[:],
                    scale=1.0,
                    accum_out=sumexp[:],
                )
                rsum = wpool.tile([128, 1], f32, tag="rsum")
                nc.vector.reciprocal(rsum[:], sumexp[:])
                nc.vector.tensor_scalar_mul(
                    w_acc[:, t * K : (t + 1) * K], exp8[:], rsum[:]
                )

            nc.tensor.matmul(
                counts_p[:],
                pk_s[:, 640:641],
                masks[NT // 2 - 1][:],
                start=False,
                stop=True,
                skip_group_check=True,
            )
            counts_s = apool.tile([1, 2 * E], f32)
            nc.vector.tensor_copy(counts_s[:], counts_p[:])

            nc.sync.dma_start(w_o.ap(), w_acc[:])
            nc.sync.dma_start(idx_o.ap(), idx_acc[:])
            nc.sync.dma_start(counts_o.ap(), counts_s[:])

    nc.compile()
    _CACHE["nc"] = nc
    return nc


def _install_trace_shim():
    """Enable NTFF profiling under axon (only used when KERNEL_TRACE=1)."""
    try:
        import types

        if "antenv.axon_hooks" in sys.modules:
            return True
        import antenv

        mod = types.ModuleType("antenv.axon_hooks")
        mod._hook = None
        mod.set_axon_ntff_profile_hook = lambda h: setattr(mod, "_hook", h)
        mod.get_axon_ntff_profile_hook = lambda: mod._hook
        sys.modules["antenv.axon_hooks"] = mod
        antenv.axon_hooks = mod
        from trn_agent_boot.trn_boot import _ntff_profile_via_ctypes

        mod._hook = _ntff_profile_via_ctypes("/opt/axon/libaxon_pjrt.so")
        from concourse import bass_utils

        bass_utils.upload_artifacts = lambda tmpdir: tmpdir
        return True
    except Exception:
        return False


def _prep_core_inputs(x_shard_f32):
    """x_shard [2048, 2048] f32 -> xhl [NT, 128, 2*NK*128] bf16.

    xhl[t, p, half*2048 + k*128 + tt] = half(x_shard[128*t + tt, 128*k + p])
    so each SBUF x-tile is [d-row partition, (half, chunk, token)] and chunk k
    of half h is the ready-to-use matmul lhsT [128 d, 128 tokens].
    """
    xh = x_shard_f32.astype(BF16)
    xl = (x_shard_f32 - xh.astype(np.float32)).astype(BF16)
    out = np.empty((NT, 128, 2 * NK * 128), BF16)
    for half, arr in enumerate((xh, xl)):
        # [NT, 128 tok, NK, 128 p] -> [NT, p, k, tok]
        r = arr.reshape(NT, 128, NK, 128).transpose(0, 3, 2, 1)
        out[:, :, half * NK * 128 : (half + 1) * NK * 128] = r.reshape(
            NT, 128, NK * 128
        )
    return out


def _prep_shared_inputs(gate_w, expert_bias):
    gw = np.ascontiguousarray(gate_w.T)  # [DM, E] f32
    gh = gw.astype(BF16)
    gl = (gw - gh.astype(np.float32)).astype(BF16)
    # [NK, 128 p, half, E]: per-chunk tile is [128 d-rows, gh | gl]
    ghl = np.stack(
        [g.reshape(NK, 128, E) for g in (gh, gl)], axis=2
    )  # [NK, 128, 2, E]
    # partition-major for one full-bandwidth DMA: [128, (k, half, e)]
    ghl = np.ascontiguousarray(ghl.transpose(1, 0, 2, 3).reshape(128, NK * 2 * E))

    pk = np.zeros((128, PK_W), BF16)
    pk[:, 2 * E + 128] = BF16(1.0)
    biasb = np.ascontiguousarray(
        np.broadcast_to(expert_bias.astype(np.float32), (128, E))
    )
    return ghl, pk, biasb


def kernel(x, gate_w, expert_bias):
    from concourse.bass_utils import run_bass_kernel_spmd

    x = np.asarray(x, np.float32)
    gate_w = np.asarray(gate_w, np.float32)
    expert_bias = np.asarray(expert_bias, np.float32)

    xf = x.reshape(B * S, DM)
    ghl, pk, biasb = _prep_shared_inputs(gate_w, expert_bias)

    in_maps = []
    for c in range(N_CORES):
        xhl = _prep_core_inputs(xf[c * TOK : (c + 1) * TOK])
        in_maps.append({"xhl": xhl, "ghl": ghl, "pk": pk, "biasb": biasb})

    nc = _build_program()

    trace = os.environ.get("KERNEL_TRACE", "") == "1"
    if trace:
        trace = _install_trace_shim()

    res = run_bass_kernel_spmd(
        nc, in_maps, core_ids=list(range(N_CORES)), trace=trace
    )
    LAST_PROFILE["exec_time_ns"] = res.exec_time_ns
    LAST_PROFILE["mean_exec_time_ns"] = res.mean_exec_time_ns
    LAST_PROFILE["trace"] = res.instructions_and_trace

    weights = np.empty((B * S, K), np.float32)
    indices = np.empty((B * S, K), np.int32)
    counts = np.zeros(E, np.float64)
    for c, out in enumerate(res.results):
        # [128 tok-in-tile, NT, K] -> [NT, 128, K] -> [2048, K]
        w = out["w_o"].reshape(128, NT, K).transpose(1, 0, 2).reshape(TOK, K)
        ix = out["idx_o"].reshape(128, NT, K).transpose(1, 0, 2).reshape(TOK, K)
        weights[c * TOK : (c + 1) * TOK] = w
        indices[c * TOK : (c + 1) * TOK] = ix.astype(np.int32)
        cc = out["counts_o"].ravel().astype(np.float64)
        counts += cc[:E] + cc[E:]

    expert_counts = counts.astype(np.float32)
    n_tokens = B * S * K
    expected_load = np.float32(n_tokens / E)
    mean = expert_counts.mean(dtype=np.float64)
    std = np.std(expert_counts.astype(np.float64), ddof=1)
    load_balance = np.float32(std / (mean + 1e-6))

    return (
        weights.reshape(B, S, K),
        indices.reshape(B, S, K),
        expert_counts,
        load_balance,
        np.float32(expert_counts.max()),
        np.float32(expert_counts.min()),
        expected_load,
    )


# revision 40
# speedup vs baseline: 1.0523x; 1.0273x over previous
"""AuxiliaryLossFreeRouter (MoE top-8 routing) on 8 Trainium2 NeuronCores.

Strategy (data-parallel over tokens, per the sharding hint):
  - 16384 tokens sharded 2048/core; gate_w + expert_bias replicated.
  - Gate matmul in bf16 hi/lo 3-pass split (xh@gh + xl@gh + xh@gl) accumulated
    in fp32 PSUM -> ~np.float32-level logits (max err ~2e-5) at 1 cycle/row
    per pass instead of fp32's 4 cycles/row. Same HBM bytes as fp32 (2x bf16).
    The xh pass streams a 512-wide rhs [gh|gl] into one PSUM bank (two
    accumulators side by side), folded by DVE adds that also apply the bias.
  - Per 128-token tile: DVE max8/max_index8 give top-8 values+indices,
    softmax over the 8 on ACT (Exp with accumulated sum) + DVE reciprocal,
    expert histogram via ones-vector matmul over a (logits >= v8) mask,
    accumulated in a dedicated PSUM bank across all tiles.
  - Host: unshard outputs, sum per-core histograms, derive scalar load stats.
"""

import os
import sys

if "/opt/trn_rl_repo" not in sys.path:
    sys.path.insert(0, "/opt/trn_rl_repo")

import ml_dtypes
import numpy as np

BF16 = ml_dtypes.bfloat16

N_CORES = 8
B, S, DM = 4, 4096, 2048
E = 256
K = 8
TOK = (B * S) // N_CORES  # 2048 tokens per core
NT = TOK // 128  # 16 token tiles per core
NK = DM // 128  # 16 contraction chunks

# packed const layout (bf16, [128, PK_W]): col 640 = ones column, the
# stationary operand of the histogram matmul (other cols unused)
PK_W = 641

_CACHE = {}
LAST_PROFILE = {}


def _build_program():
    if "nc" in _CACHE:
        return _CACHE["nc"]

    import concourse.tile as tile
    from concourse import bacc, mybir

    nc = bacc.Bacc("TRN2", target_bir_lowering=False, debug=False)

    xhl_d = nc.dram_tensor(
        "xhl", [NT, 128, 2 * NK * 128], mybir.dt.bfloat16, kind="ExternalInput"
    )
    ghl_d = nc.dram_tensor(
        "ghl", [128, NK * 2 * E], mybir.dt.bfloat16, kind="ExternalInput"
    )
    pk_d = nc.dram_tensor("pk", [128, PK_W], mybir.dt.bfloat16, kind="ExternalInput")
    biasb_d = nc.dram_tensor(
        "biasb", [128, E], mybir.dt.float32, kind="ExternalInput"
    )

    w_o = nc.dram_tensor("w_o", [128, NT * K], mybir.dt.float32, kind="ExternalOutput")
    idx_o = nc.dram_tensor(
        "idx_o", [128, NT * K], mybir.dt.uint32, kind="ExternalOutput"
    )
    counts_o = nc.dram_tensor(
        "counts_o", [1, 2 * E], mybir.dt.float32, kind="ExternalOutput"
    )

    f32 = mybir.dt.float32
    bf = mybir.dt.bfloat16

    with tile.TileContext(nc) as tc:
        with (
            tc.tile_pool(name="const", bufs=1) as cpool,
            tc.tile_pool(name="xin", bufs=6) as xpool,
            tc.tile_pool(name="work", bufs=3) as wpool,
            tc.tile_pool(name="acc", bufs=1) as apool,
            tc.tile_pool(name="lps", bufs=6, space="PSUM") as lpool,
            tc.tile_pool(name="cps", bufs=1, space="PSUM") as cpps,
        ):
            # DMA issue order = completion order on the single HWDGE queue.
            # x tile 0 goes LAST in the preamble: the PE's first matmul gates
            # on it, and by then weights + x tile 1 are resident, so the PE
            # stream never starves afterwards (a mid-stream gap re-throttles
            # the PE clock to 1.2 GHz for ~2 windows, costing far more).
            # Preamble DMA order is tuned so every transfer lands just before
            # its first consumer: weights chunks 0-7, then tile-0's xh half
            # (gates the PE start ~13us), then weight chunks 8-11 / 12-15,
            # tile-0's xl half, tile 1. Steady-state tiles stay one 1MB DMA.
            HG = NK * E  # half of the ghl columns
            QG = HG // 2
            ghl_s = cpool.tile([128, NK * 2 * E], bf)
            nc.sync.dma_start(ghl_s[:, 0:HG], ghl_d.ap()[:, 0:HG])
            g_s = [ghl_s[:, k * 2 * E : (k + 1) * 2 * E] for k in range(NK)]

            def load_x_tile(t):
                xt = xpool.tile([128, 2 * NK * 128], bf, tag="x")
                nc.sync.dma_start(xt[:], xhl_d.ap()[t])
                return xt

            XH = NK * 128
            xt0 = xpool.tile([128, 2 * NK * 128], bf, tag="x")
            nc.sync.dma_start(xt0[:, 0:XH], xhl_d.ap()[0][:, 0:XH])
            nc.sync.dma_start(ghl_s[:, HG : HG + QG], ghl_d.ap()[:, HG : HG + QG])
            nc.sync.dma_start(
                ghl_s[:, HG + QG : 2 * HG], ghl_d.ap()[:, HG + QG : 2 * HG]
            )
            nc.sync.dma_start(xt0[:, XH : 2 * XH], xhl_d.ap()[0][:, XH : 2 * XH])
            xt1 = load_x_tile(1)
            xt_pre = [xt0, xt1]

            # consts are first needed ~1 tile in; keep them off the
            # critical x/weight path
            pk_s = cpool.tile([128, PK_W], bf)
            nc.sync.dma_start(pk_s[:], pk_d.ap())
            bias_s = cpool.tile([128, E], f32)
            nc.sync.dma_start(bias_s[:], biasb_d.ap())

            w_acc = apool.tile([128, NT * K], f32)
            idx_acc = apool.tile([128, NT * K], mybir.dt.uint32)
            counts_p = cpps.tile([1, 2 * E], f32)
            masks = []

            # HAM warm-up: the PE idles ~15us waiting for the first DMAs and
            # would run the first ~3.4us of real matmuls at 1.2 GHz. Spin
            # no-dep dummy matmuls on scratch SBUF to lift the clock gate to
            # 2.4 GHz and keep it there until the real stream starts.
            warm_in = cpool.tile([128, 512], bf)
            nc.vector.memset(warm_in[:], 0.0)
            warm_p = cpps.tile([128, 512], f32, tag="warm")
            for _ in range(30):
                nc.tensor.matmul(
                    warm_p[:, 0:256],
                    warm_in[:, 0:128],
                    warm_in[:, 0:256],
                    start=True,
                    stop=True,
                )

            for t in range(NT):
                if t < len(xt_pre):
                    xt = xt_pre[t]
                else:
                    xt = load_x_tile(t)

                # xh pass with wide rhs: lp[:, 0:256] accumulates xh@gh,
                # lp[:, 256:512] accumulates xh@gl; k=0 clears the bank
                lp = lpool.tile([128, 2 * E], f32, tag="lp")
                for k in range(NK):
                    nc.tensor.matmul(
                        lp[:],
                        xt[:, k * 128 : (k + 1) * 128],
                        g_s[k][:],
                        start=(k == 0),
                        stop=False,
                    )
                # xl @ gh pass into the gh accumulator
                for k in range(NK):
                    nc.tensor.matmul(
                        lp[:, 0:E],
                        xt[:, (NK + k) * 128 : (NK + k + 1) * 128],
                        g_s[k][:, 0:E],
                        start=False,
                        stop=(k == NK - 1),
                    )

                # logits = (xh@gh + xl@gh) + (xh@gl + bias); DVE reads at
                # most one PSUM operand per op, so sum in two steps
                glpart = wpool.tile([128, E], f32, tag="glpart")
                nc.vector.tensor_add(glpart[:], bias_s[:], lp[:, E : 2 * E])
                logits = wpool.tile([128, E], f32, tag="logits")
                nc.vector.tensor_add(logits[:], glpart[:], lp[:, 0:E])

                vals8 = wpool.tile([128, 8], f32, tag="vals8")
                nc.vector.max(out=vals8[:], in_=logits[:])
                nc.vector.max_index(
                    out=idx_acc[:, t * K : (t + 1) * K],
                    in_max=vals8[:],
                    in_values=logits[:],
                )

                # two tiles' masks sit side by side in one [128, 512] tile so
                # the histogram matmul runs once per pair at N=512
                if t % 2 == 0:
                    mask2 = wpool.tile([128, 2 * E], bf, tag="mask")
                    masks.append(mask2)
                nc.vector.tensor_scalar(
                    masks[t // 2][:, (t % 2) * E : (t % 2 + 1) * E],
                    logits[:],
                    vals8[:, 7:8],
                    None,
                    op0=mybir.AluOpType.is_ge,
                )
                # the counts matmul for the previous pair: both its masks are
                # long done, so the in-order PE queue doesn't stall on the DVE
                if t >= 2 and t % 2 == 0:
                    nc.tensor.matmul(
                        counts_p[:],
                        pk_s[:, 640:641],
                        masks[t // 2 - 1][:],
                        start=(t == 2),
                        stop=False,
                        skip_group_check=True,
                    )

                negmax = wpool.tile([128, 1], f32, tag="negmax")
                nc.vector.tensor_scalar_mul(negmax[:], vals8[:, 0:1], -1.0)
                exp8 = wpool.tile([128, 8], f32, tag="exp8")
                sumexp = wpool.tile([128, 1], f32, tag="sumexp")
                nc.scalar.activation(
                    exp8[:],
                    vals8[:],
                    mybir.ActivationFunctionType.Exp,
                    bias=negmax[:],
                    scale=1.0,
                    accum_out=sumexp[:],
                )
                rsum = wpool.tile([128, 1], f32, tag="rsum")
                nc.vector.reciprocal(rsum[:], sumexp[:])
                nc.vector.tensor_scalar_mul(
                    w_acc[:, t * K : (t + 1) * K], exp8[:], rsum[:]
                )

            nc.tensor.matmul(
                counts_p[:],
                pk_s[:, 640:641],
                masks[NT // 2 - 1][:],
                start=False,
                stop=True,
                skip_group_check=True,
            )
            counts_s = apool.tile([1, 2 * E], f32)
            nc.vector.tensor_copy(counts_s[:], counts_p[:])

            nc.sync.dma_start(w_o.ap(), w_acc[:])
            nc.sync.dma_start(idx_o.ap(), idx_acc[:])
            nc.sync.dma_start(counts_o.ap(), counts_s[:])

    nc.compile()
    _CACHE["nc"] = nc
    return nc


def _install_trace_shim():
    """Enable NTFF profiling under axon (only used when KERNEL_TRACE=1)."""
    try:
        import types

        if "antenv.axon_hooks" in sys.modules:
            return True
        import antenv

        mod = types.ModuleType("antenv.axon_hooks")
        mod._hook = None
        mod.set_axon_ntff_profile_hook = lambda h: setattr(mod, "_hook", h)
        mod.get_axon_ntff_profile_hook = lambda: mod._hook
        sys.modules["antenv.axon_hooks"] = mod
        antenv.axon_hooks = mod
        from trn_agent_boot.trn_boot import _ntff_profile_via_ctypes

        mod._hook = _ntff_profile_via_ctypes("/opt/axon/libaxon_pjrt.so")
        from concourse import bass_utils

        bass_utils.upload_artifacts = lambda tmpdir: tmpdir
        return True
    except Exception:
        return False


def _prep_core_inputs(x_shard_f32):
    """x_shard [2048, 2048] f32 -> xhl [NT, 128, 2*NK*128] bf16.

    xhl[t, p, half*2048 + k*128 + tt] = half(x_shard[128*t + tt, 128*k + p])
    so each SBUF x-tile is [d-row partition, (half, chunk, token)] and chunk k
    of half h is the ready-to-use matmul lhsT [128 d, 128 tokens].
    """
    xh = x_shard_f32.astype(BF16)
    xl = (x_shard_f32 - xh.astype(np.float32)).astype(BF16)
    out = np.empty((NT, 128, 2 * NK * 128), BF16)
    for half, arr in enumerate((xh, xl)):
        # [NT, 128 tok, NK, 128 p] -> [NT, p, k, tok]
        r = arr.reshape(NT, 128, NK, 128).transpose(0, 3, 2, 1)
        out[:, :, half * NK * 128 : (half + 1) * NK * 128] = r.reshape(
            NT, 128, NK * 128
        )
    return out


def _prep_shared_inputs(gate_w, expert_bias):
    gw = np.ascontiguousarray(gate_w.T)  # [DM, E] f32
    gh = gw.astype(BF16)
    gl = (gw - gh.astype(np.float32)).astype(BF16)
    # [NK, 128 p, half, E]: per-chunk tile is [128 d-rows, gh | gl]
    ghl = np.stack(
        [g.reshape(NK, 128, E) for g in (gh, gl)], axis=2
    )  # [NK, 128, 2, E]
    # partition-major for one full-bandwidth DMA: [128, (k, half, e)]
    ghl = np.ascontiguousarray(ghl.transpose(1, 0, 2, 3).reshape(128, NK * 2 * E))

    pk = np.zeros((128, PK_W), BF16)
    pk[:, 2 * E + 128] = BF16(1.0)
    biasb = np.ascontiguousarray(
        np.broadcast_to(expert_bias.astype(np.float32), (128, E))
    )
    return ghl, pk, biasb


def kernel(x, gate_w, expert_bias):
    from concourse.bass_utils import run_bass_kernel_spmd

    x = np.asarray(x, np.float32)
    gate_w = np.asarray(gate_w, np.float32)
    expert_bias = np.asarray(expert_bias, np.float32)

    xf = x.reshape(B * S, DM)
    ghl, pk, biasb = _prep_shared_inputs(gate_w, expert_bias)

    in_maps = []
    for c in range(N_CORES):
        xhl = _prep_core_inputs(xf[c * TOK : (c + 1) * TOK])
        in_maps.append({"xhl": xhl, "ghl": ghl, "pk": pk, "biasb": biasb})

    nc = _build_program()

    trace = os.environ.get("KERNEL_TRACE", "") == "1"
    if trace:
        trace = _install_trace_shim()

    res = run_bass_kernel_spmd(
        nc, in_maps, core_ids=list(range(N_CORES)), trace=trace
    )
    LAST_PROFILE["exec_time_ns"] = res.exec_time_ns
    LAST_PROFILE["mean_exec_time_ns"] = res.mean_exec_time_ns
    LAST_PROFILE["trace"] = res.instructions_and_trace

    weights = np.empty((B * S, K), np.float32)
    indices = np.empty((B * S, K), np.int32)
    counts = np.zeros(E, np.float64)
    for c, out in enumerate(res.results):
        # [128 tok-in-tile, NT, K] -> [NT, 128, K] -> [2048, K]
        w = out["w_o"].reshape(128, NT, K).transpose(1, 0, 2).reshape(TOK, K)
        ix = out["idx_o"].reshape(128, NT, K).transpose(1, 0, 2).reshape(TOK, K)
        weights[c * TOK : (c + 1) * TOK] = w
        indices[c * TOK : (c + 1) * TOK] = ix.astype(np.int32)
        cc = out["counts_o"].ravel().astype(np.float64)
        counts += cc[:E] + cc[E:]

    expert_counts = counts.astype(np.float32)
    n_tokens = B * S * K
    expected_load = np.float32(n_tokens / E)
    mean = expert_counts.mean(dtype=np.float64)
    std = np.std(expert_counts.astype(np.float64), ddof=1)
    load_balance = np.float32(std / (mean + 1e-6))

    return (
        weights.reshape(B, S, K),
        indices.reshape(B, S, K),
        expert_counts,
        load_balance,
        np.float32(expert_counts.max()),
        np.float32(expert_counts.min()),
        expected_load,
    )
